# revision 7
# baseline (speedup 1.0000x reference)
"""ADMM DC-layer kernel for Trainium2 (8 NeuronCores, data-parallel over batch).

Strategy
--------
Batch (1024) is sharded 128 samples per core; A and the 128x128 SMW-system
factorization are replicated (host-precomputed S^-1 and H = rho*S^-1*Gblock).

Math (per core, bs=128 samples):
  c0T   = (A @ r_n)^T stacked            one-time, contraction over N
  per ADMM step:
    sol  = S^-1 c0T + H zmuT             (two accumulating 128x128 matmuls)
    wT   = rho*zmuT - sol
    xT   = relu(rT_r + A_stack-chunks^T @ wT)     (64 chunk matmuls + epilogue)
    AxT  = AT1-chunks^T @ xT-chunks       (64 accumulating matmuls)
    z/u dual updates in normal [sample, 2M] layout (one small PE transpose)

All big read-only operands (A_stack, AT1, rT_r, rT_i, H^T, Sinv^T, I) are
host-packed into ONE [128, COLS] array loaded by a single fully-contiguous
DMA: one completion semaphore means no matmul ever needs more than one sync
wait (fp32 matmuls embed the weight load, which has a single wait slot).
Host pre-transposes r_n and post-transposes x so every DMA is
contiguous-innermost.
"""
import os
import sys

sys.path.insert(0, "/opt/trn_rl_repo")

import numpy as np

BATCH = 1024
M = 64
N = 8192
STEPS = 3
NCORES = 8
BS = BATCH // NCORES  # 128 samples per core
T = N // 128          # 64 chunks along N
EPS_DIV = 1e-12

# mega layout (columns, all f32, 128 partitions)
C_ASTACK = 0            # [128, N]   [[Ar],[Ai]]
C_AT1 = N               # [128, N]   chunked [ArT|AiT]: col t*128+j -> AT1[t*128+p, j]
C_RTR = 2 * N           # [128, N]   chunked r_n real transposed: col t*BS+s
C_RTI = 3 * N           # [128, N]   chunked r_n imag transposed
C_HT = 4 * N            # [128, 128] H^T
C_SIT = 4 * N + 128     # [128, 128] Sinv^T
C_I = 4 * N + 256       # [128, 128] identity
COLS = 4 * N + 384

_BUILD_CACHE = {}


def build_bass(rho, eps):
    """Build the per-core Bass program (identical on all cores)."""
    import concourse.bass as bass
    import concourse.bacc as bacc
    import concourse.tile as tile
    import concourse.mybir as mybir

    f32 = mybir.dt.float32
    Alu = mybir.AluOpType
    Act = mybir.ActivationFunctionType

    nc = bacc.Bacc("TRN2", target_bir_lowering=False)

    mega_d = nc.dram_tensor("mega", [128, COLS], f32, kind="ExternalInput")
    y_d = nc.dram_tensor("y", [BS, 128], f32, kind="ExternalInput")
    u0_d = nc.dram_tensor("u0", [BS, 128], f32, kind="ExternalInput")

    xTo_d = nc.dram_tensor("xTo", [N, BS], f32, kind="ExternalOutput")
    uo_d = nc.dram_tensor("uo", [BS, 128], f32, kind="ExternalOutput")

    with tile.TileContext(nc) as tc:
        with (
            tc.tile_pool(name="big", bufs=1) as big,
            tc.tile_pool(name="small", bufs=1) as small,
            tc.tile_pool(name="ps", bufs=1, space="PSUM") as ps,
        ):
            mega = big.tile([128, COLS], f32, tag="mega")
            nc.sync.dma_start(mega[:], mega_d[:])

            A_stack = mega[:, C_ASTACK:C_ASTACK + N]
            AT1 = mega[:, C_AT1:C_AT1 + N]
            rTr = mega[:, C_RTR:C_RTR + N]
            rTi = mega[:, C_RTI:C_RTI + N]
            HT_sb = mega[:, C_HT:C_HT + 128]
            SiT_sb = mega[:, C_SIT:C_SIT + 128]
            I_sb = mega[:, C_I:C_I + 128]

            xT = big.tile([128, N], f32, tag="xT")

            y_sb = small.tile([BS, 128], f32, tag="y")
            nc.sync.dma_start(y_sb[:], y_d[:])
            u_sb = small.tile([BS, 128], f32, tag="u")
            nc.sync.dma_start(u_sb[:], u0_d[:])

            # ---- c0T = (A @ r_n)^T stacked [128, BS] ----
            P1 = ps.tile([128, BS], f32, tag="pc", bufs=3)
            for t in range(T):
                nc.tensor.matmul(
                    P1[:], AT1[:, t * 128:(t + 1) * 128],
                    rTr[:, t * BS:(t + 1) * BS],
                    start=(t == 0), stop=(t == T - 1),
                )
            P2 = ps.tile([128, BS], f32, tag="pc", bufs=3)
            for t in range(T):
                nc.tensor.matmul(
                    P2[:], AT1[:, t * 128:(t + 1) * 128],
                    rTi[:, t * BS:(t + 1) * BS],
                    start=(t == 0), stop=(t == T - 1),
                )
            c0T = small.tile([128, BS], f32, tag="c0T")
            p2s = small.tile([128, BS], f32, tag="p2s")
            nc.scalar.copy(p2s[:], P2[:])
            # c0_r^T = P1[:64] - P2[64:], c0_i^T = P2[:64] + P1[64:]
            nc.vector.tensor_sub(c0T[0:64, :], P1[0:64, :], p2s[64:128, :])
            nc.vector.tensor_add(c0T[64:128, :], P1[64:128, :], p2s[0:64, :])

            # ---- ADMM steps ----
            zmu = small.tile([BS, 128], f32, tag="zmu")
            nc.vector.tensor_sub(zmu[:], y_sb[:], u_sb[:])  # z0 - u0

            for step in range(STEPS):
                # zmuT
                tp1 = ps.tile([128, 128], f32, tag="tp", bufs=2)
                nc.tensor.transpose(tp1[:], zmu[:], I_sb)
                zmuT = small.tile([128, 128], f32, tag="zmuT", bufs=2)
                nc.vector.tensor_copy(zmuT[:], tp1[:])

                # sol = H @ zmuT + S^-1 @ c0T
                sol = ps.tile([128, BS], f32, tag="sol", bufs=1)
                nc.tensor.matmul(sol[:], HT_sb, zmuT[:], start=True, stop=False)
                nc.tensor.matmul(sol[:], SiT_sb, c0T[:], start=False, stop=True)

                # wT = rho * zmuT - sol
                wT = small.tile([128, 128], f32, tag="wT", bufs=2)
                nc.vector.scalar_tensor_tensor(
                    wT[:], zmuT[:], float(rho), sol[:],
                    op0=Alu.mult, op1=Alu.subtract,
                )

                # x-update + Ax accumulation, 4 chunks at a time
                ax_ps = ps.tile([128, BS], f32, tag="ax", bufs=2)
                for q in range(T // 4):
                    pc = ps.tile([128, 512], f32, tag="pc", bufs=3)
                    for i in range(4):
                        t = 4 * q + i
                        nc.tensor.matmul(
                            pc[:, i * 128:(i + 1) * 128],
                            A_stack[:, t * 128:(t + 1) * 128], wT[:],
                            start=True, stop=True,
                        )
                    sl = slice(q * 512, (q + 1) * 512)
                    nc.vector.tensor_add(xT[:, sl], pc[:], rTr[:, sl])
                    nc.scalar.activation(xT[:, sl], xT[:, sl], Act.Relu)
                    for i in range(4):
                        t = 4 * q + i
                        nc.tensor.matmul(
                            ax_ps[:], AT1[:, t * 128:(t + 1) * 128],
                            xT[:, t * 128:(t + 1) * 128],
                            start=(t == 0), stop=(t == T - 1),
                        )

                # Ax back to normal layout (copy on ACT so the ax-psum slot
                # and the xT slices are freed by the same semaphore)
                axT_sb = small.tile([128, BS], f32, tag="axT", bufs=2)
                nc.scalar.copy(axT_sb[:], ax_ps[:])
                ax_n = ps.tile([128, 128], f32, tag="tp", bufs=2)
                nc.tensor.transpose(ax_n[:], axT_sb[:], I_sb)

                # dual updates (normal layout)
                uy = small.tile([BS, 128], f32, tag="uy", bufs=2)
                nc.vector.tensor_sub(uy[:], u_sb[:], y_sb[:])
                v = small.tile([BS, 128], f32, tag="v", bufs=2)
                nc.vector.tensor_add(v[:], ax_n[:], uy[:])
                vsq = small.tile([BS, 128], f32, tag="vsq", bufs=2)
                nrm2 = small.tile([BS, 1], f32, tag="nrm2", bufs=2)
                nc.vector.scalar_tensor_tensor(
                    vsq[:], v[:], 1.0, v[:],
                    op0=Alu.mult, op1=Alu.mult, accum_out=nrm2[:],
                )
                nrm = small.tile([BS, 1], f32, tag="nrm", bufs=2)
                nc.scalar.sqrt(nrm[:], nrm2[:])
                nc.vector.tensor_scalar_add(nrm[:], nrm[:], EPS_DIV)
                rec = small.tile([BS, 1], f32, tag="rec", bufs=2)
                nc.vector.reciprocal(rec[:], nrm[:])
                fs = small.tile([BS, 1], f32, tag="fs", bufs=2)
                nc.vector.tensor_scalar(
                    fs[:], rec[:], float(eps), 1.0, op0=Alu.mult, op1=Alu.min,
                )
                # z = y + v * f
                z_new = small.tile([BS, 128], f32, tag="z", bufs=2)
                nc.vector.scalar_tensor_tensor(
                    z_new[:], v[:], fs[:], y_sb[:], op0=Alu.mult, op1=Alu.add,
                )
                # u += Ax - z
                t1 = small.tile([BS, 128], f32, tag="t1", bufs=2)
                nc.vector.tensor_sub(t1[:], ax_n[:], z_new[:])
                nc.vector.tensor_add(u_sb[:], u_sb[:], t1[:])
                if step < STEPS - 1:
                    nc.vector.tensor_sub(zmu[:], z_new[:], u_sb[:])

            # ---- stores ----
            nc.sync.dma_start(
                xTo_d[:].rearrange("(t p) s -> p t s", p=128),
                xT[:].rearrange("p (t s) -> p t s", s=BS),
            )
            nc.sync.dma_start(uo_d[:], u_sb[:])

    nc.compile()
    return nc


def _host_prep(A, log_rho, log_epsilon):
    rho = float(np.exp(np.float64(np.asarray(log_rho))))
    eps = float(np.exp(np.float64(np.asarray(log_epsilon))))
    Ar = np.asarray(A[0], np.float64)
    Ai = np.asarray(A[1], np.float64)
    Gr = Ar @ Ar.T + Ai @ Ai.T
    Gi = Ai @ Ar.T - Ar @ Ai.T
    Sr = np.eye(M) / (rho + EPS_DIV) + Gr
    Sb = np.block([[Sr, -Gi], [Gi, Sr]])
    Sinv = np.linalg.inv(Sb)
    Gb = np.block([[Gr, -Gi], [Gi, Gr]])
    H = rho * (Sinv @ Gb)
    HT = np.ascontiguousarray(H.T, dtype=np.float32)
    SinvT = np.ascontiguousarray(Sinv.T, dtype=np.float32)
    return rho, eps, HT, SinvT


def _chunked_T(rT):
    """[N, BS] -> [128, N] with col t*BS+s = rT[t*128+p, s]."""
    return np.ascontiguousarray(
        rT.reshape(T, 128, BS).transpose(1, 0, 2).reshape(128, T * BS)
    )


def make_in_maps(r_n, y, u_in, A, log_rho, log_epsilon):
    rho, eps, HT, SinvT = _host_prep(A, log_rho, log_epsilon)
    A_f = np.asarray(A, np.float32)
    A_stack = A_f.reshape(128, N)
    AT1 = np.concatenate([A_f[0].T, A_f[1].T], axis=1)  # [N, 128]
    AT1_ch = _chunked_T(AT1)
    I128 = np.eye(128, dtype=np.float32)
    r_n = np.asarray(r_n, np.float32)
    y = np.asarray(y, np.float32)
    u_in = np.asarray(u_in, np.float32)

    in_maps = []
    for c in range(NCORES):
        sl = slice(c * BS, (c + 1) * BS)
        rTr_ch = _chunked_T(np.ascontiguousarray(r_n[sl, 0, :].T))
        rTi_ch = _chunked_T(np.ascontiguousarray(r_n[sl, 1, :].T))
        mega = np.concatenate(
            [A_stack, AT1_ch, rTr_ch, rTi_ch, HT, SinvT, I128], axis=1
        )
        assert mega.shape == (128, COLS)
        in_maps.append({
            "mega": np.ascontiguousarray(mega, np.float32),
            "y": np.ascontiguousarray(y[sl].reshape(BS, 128)),
            "u0": np.ascontiguousarray(u_in[sl].reshape(BS, 128)),
        })
    return rho, eps, in_maps


def assemble_outputs(out_maps):
    xs = []
    us = []
    for c in range(NCORES):
        xs.append(np.asarray(out_maps[c]["xTo"]).T)  # [BS, N]
        us.append(np.asarray(out_maps[c]["uo"]).reshape(BS, 2, M))
    x_r = np.concatenate(xs, axis=0)  # [1024, N]
    x = np.stack([x_r, np.zeros_like(x_r)], axis=1)
    u = np.concatenate(us, axis=0)
    return x.astype(np.float32), u.astype(np.float32)


def kernel(r_n, y, u_in, A, log_rho, log_epsilon, _trace=False):
    from concourse.bass_utils import run_bass_kernel_spmd

    rho, eps, in_maps = make_in_maps(r_n, y, u_in, A, log_rho, log_epsilon)
    key = (round(rho, 12), round(eps, 12))
    if key not in _BUILD_CACHE:
        _BUILD_CACHE[key] = build_bass(rho, eps)
    nc = _BUILD_CACHE[key]
    res = run_bass_kernel_spmd(
        nc, in_maps, core_ids=list(range(NCORES)), trace=_trace,
    )
    x, u = assemble_outputs(res.results)
    if _trace:
        kernel._last_exec_time_ns = res.exec_time_ns
        kernel._last_results = res
    return x, u


# revision 9
# speedup vs baseline: 1.2081x; 1.2081x over previous
"""ADMM DC-layer kernel for Trainium2 (8 NeuronCores, data-parallel over batch).

Strategy
--------
Batch (1024) is sharded 128 samples per core; A and the 128x128 SMW-system
factorization are replicated (host-precomputed S^-1 and H = rho*S^-1*Gblock).

Math (per core, bs=128 samples):
  c0T   = (A @ r_n)^T stacked            one-time, contraction over N
  per ADMM step:
    sol  = S^-1 c0T + H zmuT             (two accumulating 128x128 matmuls)
    wT   = rho*zmuT - sol
    xT   = relu(rT_r + A_stack-chunks^T @ wT)     (64 chunk matmuls + epilogue)
    AxT  = AT1-chunks^T @ xT-chunks       (64 accumulating matmuls)
    z/u dual updates in normal [sample, 2M] layout (one small PE transpose)

Loads are host-packed, fully contiguous, and pipelined: G group slabs
(AT1 | rT_r | rT_i interleaved) stream in while the one-time c0 matmuls
consume them group by group; A_stack arrives last, just in time for step 1.
The xT store is split and overlapped with step-3 compute. Host pre-transposes
r_n and post-transposes x so every DMA is contiguous-innermost.

TRN2 gotchas honored here: at most 1 sync wait per instruction (use Bacc +
compile() for the legalization passes), no DVE op may read 2 PSUM operands,
DMA APs max 3 dims with contiguous innermost on both sides.
"""
import os
import sys

sys.path.insert(0, "/opt/trn_rl_repo")

import numpy as np

BATCH = 1024
M = 64
N = 8192
STEPS = 3
NCORES = 8
BS = BATCH // NCORES   # 128 samples per core
T = N // 128           # 64 chunks along N
G = 8                  # load groups
TG = T // G            # 8 chunks per group
GCOLS = 3 * TG * 128   # AT1 | rTr | rTi sub-blocks per group
EPS_DIV = 1e-12

_BUILD_CACHE = {}


def build_bass(rho, eps):
    """Build the per-core Bass program (identical on all cores)."""
    import concourse.bacc as bacc
    import concourse.tile as tile
    import concourse.mybir as mybir

    f32 = mybir.dt.float32
    Alu = mybir.AluOpType
    Act = mybir.ActivationFunctionType

    nc = bacc.Bacc("TRN2", target_bir_lowering=False)

    sm_d = nc.dram_tensor("sm", [128, 384], f32, kind="ExternalInput")  # HT|SinvT|I
    grp_d = [
        nc.dram_tensor(f"g{g}", [128, GCOLS], f32, kind="ExternalInput")
        for g in range(G)
    ]
    As_d = nc.dram_tensor("As", [128, N], f32, kind="ExternalInput")
    y_d = nc.dram_tensor("y", [BS, 128], f32, kind="ExternalInput")
    u0_d = nc.dram_tensor("u0", [BS, 128], f32, kind="ExternalInput")

    xTo_d = nc.dram_tensor("xTo", [N, BS], f32, kind="ExternalOutput")
    uo_d = nc.dram_tensor("uo", [BS, 128], f32, kind="ExternalOutput")

    with tile.TileContext(nc) as tc:
        with (
            tc.tile_pool(name="big", bufs=1) as big,
            tc.tile_pool(name="small", bufs=1) as small,
            tc.tile_pool(name="ps", bufs=1, space="PSUM") as ps,
        ):
            # ---- loads, in pipeline order ----
            sm = small.tile([128, 384], f32, tag="sm")
            nc.sync.dma_start(sm[:], sm_d[:])
            y_sb = small.tile([BS, 128], f32, tag="y")
            nc.sync.dma_start(y_sb[:], y_d[:])
            u_sb = small.tile([BS, 128], f32, tag="u")
            nc.sync.dma_start(u_sb[:], u0_d[:])

            grp = []
            for g in range(G):
                gt = big.tile([128, GCOLS], f32, tag=f"grp{g}")
                nc.sync.dma_start(gt[:], grp_d[g][:])
                grp.append(gt)

            A_stack = big.tile([128, N], f32, tag="A_stack")
            nc.sync.dma_start(A_stack[:], As_d[:])

            HT_sb = sm[:, 0:128]
            SiT_sb = sm[:, 128:256]
            I_sb = sm[:, 256:384]

            def at1(t):  # AT1 chunk t [128, 128]
                g, i = divmod(t, TG)
                return grp[g][:, i * 128:(i + 1) * 128]

            def rtr(t, w=1):  # rTr chunks t..t+w-1 (must stay in one group)
                g, i = divmod(t, TG)
                return grp[g][:, TG * 128 + i * 128: TG * 128 + (i + w) * 128]

            def rti(t):
                g, i = divmod(t, TG)
                return grp[g][:, 2 * TG * 128 + i * 128: 2 * TG * 128 + (i + 1) * 128]

            xT = big.tile([128, N], f32, tag="xT")

            # ---- c0T = (A @ r_n)^T stacked [128, BS], pipelined per group ----
            P1 = ps.tile([128, BS], f32, tag="pc", bufs=3)
            P2 = ps.tile([128, BS], f32, tag="pc", bufs=3)
            for g in range(G):
                for i in range(TG):
                    t = g * TG + i
                    nc.tensor.matmul(P1[:], at1(t), rtr(t),
                                     start=(t == 0), stop=(t == T - 1))
                for i in range(TG):
                    t = g * TG + i
                    nc.tensor.matmul(P2[:], at1(t), rti(t),
                                     start=(t == 0), stop=(t == T - 1))
            c0T = small.tile([128, BS], f32, tag="c0T")
            p2s = small.tile([128, BS], f32, tag="p2s")
            nc.scalar.copy(p2s[:], P2[:])
            # c0_r^T = P1[:64] - P2[64:], c0_i^T = P2[:64] + P1[64:]
            nc.vector.tensor_sub(c0T[0:64, :], P1[0:64, :], p2s[64:128, :])
            nc.vector.tensor_add(c0T[64:128, :], P1[64:128, :], p2s[0:64, :])

            # ---- ADMM steps ----
            zmu = small.tile([BS, 128], f32, tag="zmu")
            nc.vector.tensor_sub(zmu[:], y_sb[:], u_sb[:])  # z0 - u0

            for step in range(STEPS):
                last = step == STEPS - 1
                # zmuT
                tp1 = ps.tile([128, 128], f32, tag="tp", bufs=2)
                nc.tensor.transpose(tp1[:], zmu[:], I_sb)
                zmuT = small.tile([128, 128], f32, tag="zmuT", bufs=2)
                nc.vector.tensor_copy(zmuT[:], tp1[:])

                # sol = H @ zmuT + S^-1 @ c0T
                sol = ps.tile([128, BS], f32, tag="sol", bufs=1)
                nc.tensor.matmul(sol[:], HT_sb, zmuT[:], start=True, stop=False)
                nc.tensor.matmul(sol[:], SiT_sb, c0T[:], start=False, stop=True)

                # wT = rho * zmuT - sol
                wT = small.tile([128, 128], f32, tag="wT", bufs=2)
                nc.vector.scalar_tensor_tensor(
                    wT[:], zmuT[:], float(rho), sol[:],
                    op0=Alu.mult, op1=Alu.subtract,
                )

                # x-update + Ax accumulation, 4 chunks at a time
                ax_ps = ps.tile([128, BS], f32, tag="ax", bufs=2)
                for q in range(T // 4):
                    pc = ps.tile([128, 512], f32, tag="pc", bufs=3)
                    for i in range(4):
                        t = 4 * q + i
                        nc.tensor.matmul(
                            pc[:, i * 128:(i + 1) * 128],
                            A_stack[:, t * 128:(t + 1) * 128], wT[:],
                            start=True, stop=True,
                        )
                    sl = slice(q * 512, (q + 1) * 512)
                    nc.vector.tensor_add(xT[:, sl], pc[:], rtr(4 * q, 4))
                    nc.scalar.activation(xT[:, sl], xT[:, sl], Act.Relu)
                    for i in range(4):
                        t = 4 * q + i
                        nc.tensor.matmul(
                            ax_ps[:], at1(t), xT[:, t * 128:(t + 1) * 128],
                            start=(t == 0), stop=(t == T - 1),
                        )
                    if last and q % 4 == 3:
                        # overlap the output store with step-3 compute
                        lo = (q - 3) * 512
                        nc.sync.dma_start(
                            xTo_d[lo:lo + 2048, :].rearrange("(t p) s -> p t s", p=128),
                            xT[:, lo:lo + 2048].rearrange("p (t s) -> p t s", s=BS),
                        )

                # Ax back to normal layout (copy on ACT so the ax-psum slot
                # and the xT slices are freed by the same semaphore)
                axT_sb = small.tile([128, BS], f32, tag="axT", bufs=2)
                nc.scalar.copy(axT_sb[:], ax_ps[:])
                ax_n = ps.tile([128, 128], f32, tag="tp", bufs=2)
                nc.tensor.transpose(ax_n[:], axT_sb[:], I_sb)

                # dual updates (normal layout)
                uy = small.tile([BS, 128], f32, tag="uy", bufs=2)
                nc.vector.tensor_sub(uy[:], u_sb[:], y_sb[:])
                v = small.tile([BS, 128], f32, tag="v", bufs=2)
                nc.vector.tensor_add(v[:], ax_n[:], uy[:])
                vsq = small.tile([BS, 128], f32, tag="vsq", bufs=2)
                nrm2 = small.tile([BS, 1], f32, tag="nrm2", bufs=2)
                nc.vector.scalar_tensor_tensor(
                    vsq[:], v[:], 1.0, v[:],
                    op0=Alu.mult, op1=Alu.mult, accum_out=nrm2[:],
                )
                nrm = small.tile([BS, 1], f32, tag="nrm", bufs=2)
                nc.scalar.sqrt(nrm[:], nrm2[:])
                nc.vector.tensor_scalar_add(nrm[:], nrm[:], EPS_DIV)
                rec = small.tile([BS, 1], f32, tag="rec", bufs=2)
                nc.vector.reciprocal(rec[:], nrm[:])
                fs = small.tile([BS, 1], f32, tag="fs", bufs=2)
                nc.vector.tensor_scalar(
                    fs[:], rec[:], float(eps), 1.0, op0=Alu.mult, op1=Alu.min,
                )
                # z = y + v * f
                z_new = small.tile([BS, 128], f32, tag="z", bufs=2)
                nc.vector.scalar_tensor_tensor(
                    z_new[:], v[:], fs[:], y_sb[:], op0=Alu.mult, op1=Alu.add,
                )
                # u += Ax - z
                t1 = small.tile([BS, 128], f32, tag="t1", bufs=2)
                nc.vector.tensor_sub(t1[:], ax_n[:], z_new[:])
                nc.vector.tensor_add(u_sb[:], u_sb[:], t1[:])
                if not last:
                    nc.vector.tensor_sub(zmu[:], z_new[:], u_sb[:])

            nc.sync.dma_start(uo_d[:], u_sb[:])

    nc.compile()
    return nc


def _host_prep(A, log_rho, log_epsilon):
    rho = float(np.exp(np.float64(np.asarray(log_rho))))
    eps = float(np.exp(np.float64(np.asarray(log_epsilon))))
    Ar = np.asarray(A[0], np.float64)
    Ai = np.asarray(A[1], np.float64)
    Gr = Ar @ Ar.T + Ai @ Ai.T
    Gi = Ai @ Ar.T - Ar @ Ai.T
    Sr = np.eye(M) / (rho + EPS_DIV) + Gr
    Sb = np.block([[Sr, -Gi], [Gi, Sr]])
    Sinv = np.linalg.inv(Sb)
    Gb = np.block([[Gr, -Gi], [Gi, Gr]])
    H = rho * (Sinv @ Gb)
    HT = np.ascontiguousarray(H.T, dtype=np.float32)
    SinvT = np.ascontiguousarray(Sinv.T, dtype=np.float32)
    return rho, eps, HT, SinvT


def _chunked_T(rT):
    """[N, BS] -> [128, N] with col t*BS+s = rT[t*128+p, s]."""
    return np.ascontiguousarray(
        rT.reshape(T, 128, BS).transpose(1, 0, 2).reshape(128, T * BS)
    )


def make_in_maps(r_n, y, u_in, A, log_rho, log_epsilon):
    rho, eps, HT, SinvT = _host_prep(A, log_rho, log_epsilon)
    A_f = np.asarray(A, np.float32)
    A_stack = np.ascontiguousarray(A_f.reshape(128, N))
    AT1_ch = _chunked_T(np.concatenate([A_f[0].T, A_f[1].T], axis=1))
    I128 = np.eye(128, dtype=np.float32)
    sm = np.ascontiguousarray(
        np.concatenate([HT, SinvT, I128], axis=1), np.float32
    )
    r_n = np.asarray(r_n, np.float32)
    y = np.asarray(y, np.float32)
    u_in = np.asarray(u_in, np.float32)

    in_maps = []
    for c in range(NCORES):
        sl = slice(c * BS, (c + 1) * BS)
        rTr_ch = _chunked_T(np.ascontiguousarray(r_n[sl, 0, :].T))
        rTi_ch = _chunked_T(np.ascontiguousarray(r_n[sl, 1, :].T))
        im = {
            "sm": sm,
            "As": A_stack,
            "y": np.ascontiguousarray(y[sl].reshape(BS, 128)),
            "u0": np.ascontiguousarray(u_in[sl].reshape(BS, 128)),
        }
        W = TG * 128
        for g in range(G):
            gs = slice(g * W, (g + 1) * W)
            im[f"g{g}"] = np.ascontiguousarray(np.concatenate(
                [AT1_ch[:, gs], rTr_ch[:, gs], rTi_ch[:, gs]], axis=1
            ))
        in_maps.append(im)
    return rho, eps, in_maps


def assemble_outputs(out_maps):
    xs = []
    us = []
    for c in range(NCORES):
        xs.append(np.asarray(out_maps[c]["xTo"]).T)  # [BS, N]
        us.append(np.asarray(out_maps[c]["uo"]).reshape(BS, 2, M))
    x_r = np.concatenate(xs, axis=0)  # [1024, N]
    x = np.stack([x_r, np.zeros_like(x_r)], axis=1)
    u = np.concatenate(us, axis=0)
    return x.astype(np.float32), u.astype(np.float32)


def kernel(r_n, y, u_in, A, log_rho, log_epsilon, _trace=False):
    from concourse.bass_utils import run_bass_kernel_spmd

    rho, eps, in_maps = make_in_maps(r_n, y, u_in, A, log_rho, log_epsilon)
    key = (round(rho, 12), round(eps, 12))
    if key not in _BUILD_CACHE:
        _BUILD_CACHE[key] = build_bass(rho, eps)
    nc = _BUILD_CACHE[key]
    res = run_bass_kernel_spmd(
        nc, in_maps, core_ids=list(range(NCORES)), trace=_trace,
    )
    x, u = assemble_outputs(res.results)
    if _trace:
        kernel._last_exec_time_ns = res.exec_time_ns
        kernel._last_results = res
    return x, u


# revision 11
# speedup vs baseline: 1.2939x; 1.0710x over previous
"""ADMM DC-layer kernel for Trainium2 (8 NeuronCores, data-parallel over batch).

Strategy
--------
Batch (1024) is sharded 128 samples per core; A and the 128x128 SMW-system
factorization are replicated (host-precomputed S^-1 and H = rho*S^-1*Gblock).

Math (per core, bs=128 samples):
  c0T   = (A @ r_n)^T stacked            one-time, contraction over N
  per ADMM step:
    sol  = S^-1 c0T + H zmuT             (two accumulating 128x128 matmuls)
    wT   = rho*zmuT - sol
    xT   = relu(rT_r + A_stack-chunks^T @ wT)     (64 chunk matmuls + epilogue)
    AxT  = AT1-chunks^T @ xT-chunks       (64 accumulating matmuls)
    z/u dual updates in normal [sample, 2M] layout (one small PE transpose)

Loads are host-packed, fully contiguous, and pipelined: G group slabs
(AT1 | rT_r | rT_i interleaved) stream in while the one-time c0 matmuls
consume them group by group; A_stack arrives last, just in time for step 1.
The xT store is split and overlapped with step-3 compute. Host pre-transposes
r_n and post-transposes x so every DMA is contiguous-innermost.

TRN2 gotchas honored here: at most 1 sync wait per instruction (use Bacc +
compile() for the legalization passes), no DVE op may read 2 PSUM operands,
DMA APs max 3 dims with contiguous innermost on both sides.
"""
import os
import sys

sys.path.insert(0, "/opt/trn_rl_repo")

import numpy as np

BATCH = 1024
M = 64
N = 8192
STEPS = 3
NCORES = 8
BS = BATCH // NCORES   # 128 samples per core
T = N // 128           # 64 chunks along N
G = 8                  # load groups
TG = T // G            # 8 chunks per group
GCOLS = 3 * TG * 128   # AT1 | rTr | rTi sub-blocks per group
EPS_DIV = 1e-12

_BUILD_CACHE = {}


def build_bass(rho, eps):
    """Build the per-core Bass program (identical on all cores)."""
    import concourse.bacc as bacc
    import concourse.tile as tile
    import concourse.mybir as mybir

    f32 = mybir.dt.float32
    Alu = mybir.AluOpType
    Act = mybir.ActivationFunctionType

    nc = bacc.Bacc("TRN2", target_bir_lowering=False)

    sm_d = nc.dram_tensor("sm", [128, 384], f32, kind="ExternalInput")  # HT|SinvT|I
    grp_d = [
        nc.dram_tensor(f"g{g}", [128, GCOLS], f32, kind="ExternalInput")
        for g in range(G)
    ]
    As_d = [
        nc.dram_tensor("As0", [128, N // 2], f32, kind="ExternalInput"),
        nc.dram_tensor("As1", [128, N // 2], f32, kind="ExternalInput"),
    ]
    y_d = nc.dram_tensor("y", [BS, 128], f32, kind="ExternalInput")
    u0_d = nc.dram_tensor("u0", [BS, 128], f32, kind="ExternalInput")

    xTo_d = nc.dram_tensor("xTo", [N, BS], f32, kind="ExternalOutput")
    uo_d = nc.dram_tensor("uo", [BS, 128], f32, kind="ExternalOutput")

    with tile.TileContext(nc) as tc:
        with (
            tc.tile_pool(name="big", bufs=1) as big,
            tc.tile_pool(name="small", bufs=1) as small,
            tc.tile_pool(name="ps", bufs=1, space="PSUM") as ps,
        ):
            # ---- loads, in pipeline order ----
            sm = small.tile([128, 384], f32, tag="sm")
            nc.sync.dma_start(sm[:], sm_d[:])
            y_sb = small.tile([BS, 128], f32, tag="y")
            nc.sync.dma_start(y_sb[:], y_d[:])
            u_sb = small.tile([BS, 128], f32, tag="u")
            nc.sync.dma_start(u_sb[:], u0_d[:])

            grp = []
            for g in range(G):
                gt = big.tile([128, GCOLS], f32, tag=f"grp{g}")
                nc.sync.dma_start(gt[:], grp_d[g][:])
                grp.append(gt)

            A_stack = big.tile([128, N], f32, tag="A_stack")
            nc.sync.dma_start(A_stack[:, 0:N // 2], As_d[0][:])
            nc.sync.dma_start(A_stack[:, N // 2:N], As_d[1][:])

            HT_sb = sm[:, 0:128]
            SiT_sb = sm[:, 128:256]
            I_sb = sm[:, 256:384]

            def at1(t):  # AT1 chunk t [128, 128]
                g, i = divmod(t, TG)
                return grp[g][:, i * 128:(i + 1) * 128]

            def rtr(t, w=1):  # rTr chunks t..t+w-1 (must stay in one group)
                g, i = divmod(t, TG)
                return grp[g][:, TG * 128 + i * 128: TG * 128 + (i + w) * 128]

            def rti(t):
                g, i = divmod(t, TG)
                return grp[g][:, 2 * TG * 128 + i * 128: 2 * TG * 128 + (i + 1) * 128]

            xT = big.tile([128, N], f32, tag="xT")

            # hoisted step-1 preamble: zmuT and the H-part of sol only need
            # y, u, HT — they run during the load/c0 phase
            zmu = small.tile([BS, 128], f32, tag="zmu")
            nc.vector.tensor_sub(zmu[:], y_sb[:], u_sb[:])  # z0 - u0
            tp1 = ps.tile([128, 128], f32, tag="tp", bufs=2)
            nc.tensor.transpose(tp1[:], zmu[:], I_sb)
            zmuT = small.tile([128, 128], f32, tag="zmuT", bufs=2)
            nc.vector.tensor_copy(zmuT[:], tp1[:])
            sol = ps.tile([128, BS], f32, tag="sol", bufs=1)
            nc.tensor.matmul(sol[:], HT_sb, zmuT[:], start=True, stop=False)

            # ---- c0T = (A @ r_n)^T stacked [128, BS], pipelined per group ----
            P1 = ps.tile([128, BS], f32, tag="pc", bufs=3)
            P2 = ps.tile([128, BS], f32, tag="pc", bufs=3)
            for g in range(G):
                for i in range(TG):
                    t = g * TG + i
                    nc.tensor.matmul(P2[:], at1(t), rti(t),
                                     start=(t == 0), stop=(t == T - 1))
                for i in range(TG):
                    t = g * TG + i
                    nc.tensor.matmul(P1[:], at1(t), rtr(t),
                                     start=(t == 0), stop=(t == T - 1))
            c0T = small.tile([128, BS], f32, tag="c0T")
            p2s = small.tile([128, BS], f32, tag="p2s")
            nc.scalar.copy(p2s[:], P2[:])
            # c0_r^T = P1[:64] - P2[64:], c0_i^T = P2[:64] + P1[64:]
            nc.vector.tensor_sub(c0T[0:64, :], P1[0:64, :], p2s[64:128, :])
            nc.vector.tensor_add(c0T[64:128, :], P1[64:128, :], p2s[0:64, :])

            # ---- ADMM steps ----
            for step in range(STEPS):
                last = step == STEPS - 1
                # finish sol = H @ zmuT + S^-1 @ c0T (H-part already queued)
                nc.tensor.matmul(sol[:], SiT_sb, c0T[:], start=False, stop=True)

                # wT = rho * zmuT - sol
                wT = small.tile([128, 128], f32, tag="wT", bufs=2)
                nc.vector.scalar_tensor_tensor(
                    wT[:], zmuT[:], float(rho), sol[:],
                    op0=Alu.mult, op1=Alu.subtract,
                )

                # x-update + Ax accumulation, 4 chunks at a time
                ax_ps = ps.tile([128, BS], f32, tag="ax", bufs=2)
                for q in range(T // 4):
                    pc = ps.tile([128, 512], f32, tag="pc", bufs=3)
                    for i in range(4):
                        t = 4 * q + i
                        nc.tensor.matmul(
                            pc[:, i * 128:(i + 1) * 128],
                            A_stack[:, t * 128:(t + 1) * 128], wT[:],
                            start=True, stop=True,
                        )
                    sl = slice(q * 512, (q + 1) * 512)
                    nc.vector.tensor_add(xT[:, sl], pc[:], rtr(4 * q, 4))
                    nc.scalar.activation(xT[:, sl], xT[:, sl], Act.Relu)
                    for i in range(4):
                        t = 4 * q + i
                        nc.tensor.matmul(
                            ax_ps[:], at1(t), xT[:, t * 128:(t + 1) * 128],
                            start=(t == 0), stop=(t == T - 1),
                        )
                    if last and q % 4 == 3:
                        # overlap the output store with step-3 compute
                        lo = (q - 3) * 512
                        nc.sync.dma_start(
                            xTo_d[lo:lo + 2048, :].rearrange("(t p) s -> p t s", p=128),
                            xT[:, lo:lo + 2048].rearrange("p (t s) -> p t s", s=BS),
                        )

                # Ax back to normal layout (copy on ACT so the ax-psum slot
                # and the xT slices are freed by the same semaphore)
                axT_sb = small.tile([128, BS], f32, tag="axT", bufs=2)
                nc.scalar.copy(axT_sb[:], ax_ps[:])
                ax_n = ps.tile([128, 128], f32, tag="tp", bufs=2)
                nc.tensor.transpose(ax_n[:], axT_sb[:], I_sb)

                # dual updates (normal layout)
                uy = small.tile([BS, 128], f32, tag="uy", bufs=2)
                nc.vector.tensor_sub(uy[:], u_sb[:], y_sb[:])
                v = small.tile([BS, 128], f32, tag="v", bufs=2)
                nc.vector.tensor_add(v[:], ax_n[:], uy[:])
                vsq = small.tile([BS, 128], f32, tag="vsq", bufs=2)
                nrm2 = small.tile([BS, 1], f32, tag="nrm2", bufs=2)
                nc.vector.scalar_tensor_tensor(
                    vsq[:], v[:], 1.0, v[:],
                    op0=Alu.mult, op1=Alu.mult, accum_out=nrm2[:],
                )
                nrm = small.tile([BS, 1], f32, tag="nrm", bufs=2)
                nc.scalar.sqrt(nrm[:], nrm2[:])
                nc.vector.tensor_scalar_add(nrm[:], nrm[:], EPS_DIV)
                rec = small.tile([BS, 1], f32, tag="rec", bufs=2)
                nc.vector.reciprocal(rec[:], nrm[:])
                fs = small.tile([BS, 1], f32, tag="fs", bufs=2)
                nc.vector.tensor_scalar(
                    fs[:], rec[:], float(eps), 1.0, op0=Alu.mult, op1=Alu.min,
                )
                # z = y + v * f
                z_new = small.tile([BS, 128], f32, tag="z", bufs=2)
                nc.vector.scalar_tensor_tensor(
                    z_new[:], v[:], fs[:], y_sb[:], op0=Alu.mult, op1=Alu.add,
                )
                # u += Ax - z
                t1 = small.tile([BS, 128], f32, tag="t1", bufs=2)
                nc.vector.tensor_sub(t1[:], ax_n[:], z_new[:])
                nc.vector.tensor_add(u_sb[:], u_sb[:], t1[:])
                if not last:
                    # rolling preamble for the next step
                    nc.vector.tensor_sub(zmu[:], z_new[:], u_sb[:])
                    tp1 = ps.tile([128, 128], f32, tag="tp", bufs=2)
                    nc.tensor.transpose(tp1[:], zmu[:], I_sb)
                    zmuT = small.tile([128, 128], f32, tag="zmuT", bufs=2)
                    nc.vector.tensor_copy(zmuT[:], tp1[:])
                    sol = ps.tile([128, BS], f32, tag="sol", bufs=1)
                    nc.tensor.matmul(sol[:], HT_sb, zmuT[:], start=True, stop=False)

            nc.sync.dma_start(uo_d[:], u_sb[:])

    nc.compile()
    return nc


def _host_prep(A, log_rho, log_epsilon):
    rho = float(np.exp(np.float64(np.asarray(log_rho))))
    eps = float(np.exp(np.float64(np.asarray(log_epsilon))))
    Ar = np.asarray(A[0], np.float64)
    Ai = np.asarray(A[1], np.float64)
    Gr = Ar @ Ar.T + Ai @ Ai.T
    Gi = Ai @ Ar.T - Ar @ Ai.T
    Sr = np.eye(M) / (rho + EPS_DIV) + Gr
    Sb = np.block([[Sr, -Gi], [Gi, Sr]])
    Sinv = np.linalg.inv(Sb)
    Gb = np.block([[Gr, -Gi], [Gi, Gr]])
    H = rho * (Sinv @ Gb)
    HT = np.ascontiguousarray(H.T, dtype=np.float32)
    SinvT = np.ascontiguousarray(Sinv.T, dtype=np.float32)
    return rho, eps, HT, SinvT


def _chunked_T(rT):
    """[N, BS] -> [128, N] with col t*BS+s = rT[t*128+p, s]."""
    return np.ascontiguousarray(
        rT.reshape(T, 128, BS).transpose(1, 0, 2).reshape(128, T * BS)
    )


def make_in_maps(r_n, y, u_in, A, log_rho, log_epsilon):
    rho, eps, HT, SinvT = _host_prep(A, log_rho, log_epsilon)
    A_f = np.asarray(A, np.float32)
    A_stack = np.ascontiguousarray(A_f.reshape(128, N))
    AT1_ch = _chunked_T(np.concatenate([A_f[0].T, A_f[1].T], axis=1))
    I128 = np.eye(128, dtype=np.float32)
    sm = np.ascontiguousarray(
        np.concatenate([HT, SinvT, I128], axis=1), np.float32
    )
    r_n = np.asarray(r_n, np.float32)
    y = np.asarray(y, np.float32)
    u_in = np.asarray(u_in, np.float32)

    in_maps = []
    for c in range(NCORES):
        sl = slice(c * BS, (c + 1) * BS)
        rTr_ch = _chunked_T(np.ascontiguousarray(r_n[sl, 0, :].T))
        rTi_ch = _chunked_T(np.ascontiguousarray(r_n[sl, 1, :].T))
        im = {
            "sm": sm,
            "As0": np.ascontiguousarray(A_stack[:, :N // 2]),
            "As1": np.ascontiguousarray(A_stack[:, N // 2:]),
            "y": np.ascontiguousarray(y[sl].reshape(BS, 128)),
            "u0": np.ascontiguousarray(u_in[sl].reshape(BS, 128)),
        }
        W = TG * 128
        for g in range(G):
            gs = slice(g * W, (g + 1) * W)
            im[f"g{g}"] = np.ascontiguousarray(np.concatenate(
                [AT1_ch[:, gs], rTr_ch[:, gs], rTi_ch[:, gs]], axis=1
            ))
        in_maps.append(im)
    return rho, eps, in_maps


def assemble_outputs(out_maps):
    xs = []
    us = []
    for c in range(NCORES):
        xs.append(np.asarray(out_maps[c]["xTo"]).T)  # [BS, N]
        us.append(np.asarray(out_maps[c]["uo"]).reshape(BS, 2, M))
    x_r = np.concatenate(xs, axis=0)  # [1024, N]
    x = np.stack([x_r, np.zeros_like(x_r)], axis=1)
    u = np.concatenate(us, axis=0)
    return x.astype(np.float32), u.astype(np.float32)


def kernel(r_n, y, u_in, A, log_rho, log_epsilon, _trace=False):
    from concourse.bass_utils import run_bass_kernel_spmd

    rho, eps, in_maps = make_in_maps(r_n, y, u_in, A, log_rho, log_epsilon)
    key = (round(rho, 12), round(eps, 12))
    if key not in _BUILD_CACHE:
        _BUILD_CACHE[key] = build_bass(rho, eps)
    nc = _BUILD_CACHE[key]
    res = run_bass_kernel_spmd(
        nc, in_maps, core_ids=list(range(NCORES)), trace=_trace,
    )
    x, u = assemble_outputs(res.results)
    if _trace:
        kernel._last_exec_time_ns = res.exec_time_ns
        kernel._last_results = res
    return x, u
